# revision 16
# baseline (speedup 1.0000x reference)
"""Trainium2 Bass kernel for causal self-attention with RoPE (fp16 redesign).

Problem: x [1, 4096, 1024], W_qkv [3072, 1024], W_o [1024, 1024], fp32.
  qkv = x @ W_qkv.T; split Q,K,V into 16 heads of d_k=64; RoPE on Q,K;
  causal softmax(Q K^T / 8) @ V; concat heads; @ W_o.T.

Sharding: 2 heads per core across 8 cores (tensor parallel on the head dim).
Each core computes a full [4096, 1024] partial of the output projection in
fp16; host sums the 8 partials.

Layout/engine choices (driven by the TimelineSim cost model, where a matmul
costs out-free-size x cycles_per_row independent of contraction size):
  - everything fp16 (1.0 cycles/row at any width, half the DMA bytes,
    2x DVE modes on all-SBUF 2-byte ops)
  - RoPE rotate-half via one +-1 block-diag permutation matmul per chunk
    (rot(q) = P^T q), killing the two extra "rotated weight" projections
  - attention-out computed TRANSPOSED: aoT[q, d] with lhsT = exp tile,
    rhs = V' -- full 128 output partitions (vs 65 in the [d, q] orientation)
    halves the PV matmul cost, and the softmax denominator (ones column of
    V') lands per-q-PARTITION so normalization is a per-partition scalar mul
  - all four 128-wide q-subblocks x 2 heads of a chunk accumulate into TWO
    psum banks: sub-ranges of a bank share one accumulation group via the
    pending-zero first-touch semantics (start only on the first matmul into
    the bank, stop on the very last)
  - causal masking: only the true diagonal 128x128 staircase blocks get a
    mask multiply, on the otherwise-idle GPSIMD (Pool) engine
  - DMAs merged via multi-dim access patterns (~4 per chunk) because each
    dma_start serializes ~625ns on the single shared HWDGE device
  - projections / W_o / normalization woven into the exp-bound attention
    loop as fine-grained filler items (PE keeps busy while ACT exps)
"""
import numpy as np

import concourse.bacc as bacc
import concourse.mybir as mybir
import concourse.tile as tile
from concourse.bass_utils import run_bass_kernel_spmd

F32 = mybir.dt.float32
F16 = mybir.dt.float16
AF = mybir.ActivationFunctionType

D_MODEL = 1024
N_HEADS = 16
D_K = 64
S = 4096
N_CORES = 8
HPC = N_HEADS // N_CORES      # heads per core = 2
EPC = HPC * D_K               # head dims per core = 128
CH = 512                      # q chunk width
NCH = S // CH                 # 8 chunks
NDT = D_MODEL // 128          # 8 contraction tiles
NKB = S // 128                # 32 key blocks
ROPE_BASE = 10000.0


def _rope_tables():
    inv_freq = (1.0 / (ROPE_BASE ** (np.arange(0, D_K, 2, dtype=np.float64) / D_K)))
    t = np.arange(S, dtype=np.float64)
    freqs = np.outer(t, inv_freq)                              # [S, 32]
    cos = np.concatenate([np.cos(freqs), np.cos(freqs)], 1)    # [S, 64]
    sin = np.concatenate([np.sin(freqs), np.sin(freqs)], 1)
    cosT = np.tile(cos.T, (HPC, 1))                            # [128, S]
    sinT = np.tile(sin.T, (HPC, 1))
    return np.ascontiguousarray(
        np.concatenate([cosT, sinT], axis=1).astype(np.float16))  # [128, 2S]


def _perm_matrix():
    """P with (P^T q)[m] = rot_half(q)[m] per 64-row head block."""
    p = np.zeros((128, 128), dtype=np.float16)
    for h in range(HPC):
        for m in range(D_K):
            if m < 32:
                p[D_K * h + m + 32, D_K * h + m] = -1.0
            else:
                p[D_K * h + m - 32, D_K * h + m] = 1.0
    return p


def _mask2():
    """Staircase valid = (qq >= kk), duplicated for the 2-head strided AP."""
    kk = np.arange(128)[:, None]
    qq = np.arange(128)[None, :]
    m = (qq >= kk).astype(np.float16)
    return np.ascontiguousarray(np.concatenate([m, m], axis=1))  # [128, 256]


def _build_program():
    nc = bacc.Bacc("TRN2", target_bir_lowering=False, debug=False)

    xt_d = nc.dram_tensor("xt", [D_MODEL, S], F16, kind="ExternalInput").ap()
    wt_d = nc.dram_tensor("wt", [D_MODEL, 384], F16, kind="ExternalInput").ap()
    wot_d = nc.dram_tensor("wot", [EPC, D_MODEL], F16, kind="ExternalInput").ap()
    cs_d = nc.dram_tensor("cs", [EPC, 2 * S], F16, kind="ExternalInput").ap()
    pm_d = nc.dram_tensor("pm", [128, 128], F16, kind="ExternalInput").ap()
    id_d = nc.dram_tensor("ident", [128, 128], F16, kind="ExternalInput").ap()
    msk_d = nc.dram_tensor("msk2", [128, 256], F16, kind="ExternalInput").ap()
    y_d = nc.dram_tensor("y", [S, D_MODEL], F16, kind="ExternalOutput").ap()

    with tile.TileContext(nc) as tc:
        with tc.tile_pool(name="const", bufs=1) as cst, \
             tc.tile_pool(name="xts", bufs=2) as xtp, \
             tc.tile_pool(name="cs", bufs=2) as csp, \
             tc.tile_pool(name="qt", bufs=2) as qtp, \
             tc.tile_pool(name="qraw", bufs=2) as qrp, \
             tc.tile_pool(name="rt", bufs=2) as rtp, \
             tc.tile_pool(name="vt", bufs=2) as vtp, \
             tc.tile_pool(name="et", bufs=5) as etp, \
             tc.tile_pool(name="rc", bufs=4) as rcp, \
             tc.tile_pool(name="aot1", bufs=26) as a1p, \
             tc.tile_pool(name="aot2", bufs=8) as a2p, \
             tc.tile_pool(name="ysb", bufs=5) as ysp, \
             tc.tile_pool(name="pj_ps", bufs=2, space="PSUM") as pjp, \
             tc.tile_pool(name="sc_ps", bufs=2, space="PSUM") as scp, \
             tc.tile_pool(name="ao_ps", bufs=2, space="PSUM") as aop:

            # ---- persistent SBUF ----
            wsb = cst.tile([128, NDT * 384], F16, tag="wsb")     # qkv weights
            kt_s = cst.tile([EPC, S], F16, tag="kt")             # rope'd K
            vp_s = [cst.tile([128, 2 * (D_K + 1)], F16, tag=f"vp{i}", name=f"vp{i}")
                    for i in range(NKB)]                          # V' transposed
            pm_s = cst.tile([128, 128], F16, tag="pm")
            id_s = cst.tile([128, 128], F16, tag="id")
            msk_s = cst.tile([128, 256], F16, tag="msk")
            wot_s = cst.tile([EPC, D_MODEL], F16, tag="wot")

            # first DMAs, ordered so the chunk-0 Q projection can start as
            # early as possible: first x/W halves, then the rest
            xts0 = xtp.tile([128, NDT * CH], F16, tag="xts", name="xts0")
            half = NDT // 2
            nc.sync.dma_start(
                xts0[:, 0:half * CH].rearrange("p (dt c) -> p dt c", dt=half),
                xt_d[0:half * 128, 0:CH].rearrange("(dt p) c -> p dt c", dt=half))
            nc.sync.dma_start(
                wsb[:, 0:half * 384].rearrange("p (dt c) -> p dt c", dt=half),
                wt_d[0:half * 128, :].rearrange("(dt p) c -> p dt c", dt=half))
            nc.sync.dma_start(
                xts0[:, half * CH:].rearrange("p (dt c) -> p dt c", dt=half),
                xt_d[half * 128:, 0:CH].rearrange("(dt p) c -> p dt c", dt=half))
            nc.sync.dma_start(
                wsb[:, half * 384:].rearrange("p (dt c) -> p dt c", dt=half),
                wt_d[half * 128:, :].rearrange("(dt p) c -> p dt c", dt=half))
            cs0 = csp.tile([128, 2 * CH], F16, tag="cs", name="cs0")
            nc.sync.dma_start(
                cs0[:].rearrange("p (t c) -> p t c", t=2),
                cs_d[:].rearrange("p (t c) -> p t c", t=2)[:, :, 0:CH])
            nc.sync.dma_start(pm_s[:], pm_d[:])
            nc.sync.dma_start(id_s[:], id_d[:])
            nc.sync.dma_start(msk_s[:], msk_d[:])
            nc.sync.dma_start(wot_s[:], wot_d[:])
            # ones columns of V' (cols 64 and 129), written once
            for i in range(NKB):
                nc.gpsimd.memset(
                    vp_s[i][:].rearrange("p (h c) -> p h c", h=2)[:, :, D_K:D_K + 1],
                    1.0)

            filler = []   # next-chunk Q projection work (due by chunk end)
            kvq = []      # this chunk's K/V work (due before its diagonal)
            normq = []    # psum-bank normalizations (run promptly)
            woq = []      # deferred Wo/output backlog (spent in late chunks)
            xcs = {}      # chunk -> (xts tile, cs tile)

            def push_dma_now(m):
                xts_t = xtp.tile([128, NDT * CH], F16, tag="xts", name=f"xts{m}")
                sl = slice(CH * m, CH * (m + 1))
                nc.sync.dma_start(
                    xts_t[:].rearrange("p (dt c) -> p dt c", dt=NDT),
                    xt_d[:, sl].rearrange("(dt p) c -> p dt c", dt=NDT))
                cs_t = csp.tile([128, 2 * CH], F16, tag="cs", name=f"cs{m}")
                nc.sync.dma_start(
                    cs_t[:].rearrange("p (t c) -> p t c", t=2),
                    cs_d[:].rearrange("p (t c) -> p t c", t=2)[:, :, sl])
                xcs[m] = (xts_t, cs_t)

            def proj_items(q, m, b, st, key):
                """Per-matmul projection items; only the group-closing item is
                a safe stopping point for pull(), so deferred Wo work never
                interleaves into an open psum accumulation group."""
                def mk(dt_i):
                    def go():
                        if dt_i == 0:
                            st[key] = pjp.tile([128, CH], F32, tag="pj",
                                               name=f"pj{m}_{b}")
                        nc.tensor.matmul(
                            st[key][:],
                            wsb[:, 384 * dt_i + 128 * b:384 * dt_i + 128 * (b + 1)],
                            xcs[m][0][:, CH * dt_i:CH * (dt_i + 1)],
                            start=(dt_i == 0), stop=(dt_i == NDT - 1))
                    return go
                for dt_i in range(NDT):
                    q.append((mk(dt_i), dt_i == NDT - 1))

            def push_q(m, qt_t):
                """x/cos-sin DMA + Q projection + RoPE for chunk m."""
                filler.append((lambda: push_dma_now(m), True))
                st = {}
                proj_items(filler, m, 0, st, "ps")

                def rope_a():
                    raw = qrp.tile([128, CH], F16, tag="qraw", name=f"qr{m}_q")
                    nc.vector.tensor_copy(raw[:], st["ps"][:])
                    pr = pjp.tile([128, CH], F32, tag="pj", name=f"prm{m}_q")
                    nc.tensor.matmul(pr[:], pm_s[:], raw[:], start=True, stop=True)
                    st["raw"], st["pr"] = raw, pr
                filler.append((rope_a, True))

                def rope_b():
                    cs_t = xcs[m][1]
                    nc.vector.tensor_mul(qt_t[:], st["raw"][:], cs_t[:, 0:CH])
                    rt = rtp.tile([128, CH], F16, tag="rt")
                    nc.vector.tensor_mul(rt[:], st["pr"][:], cs_t[:, CH:2 * CH])
                    nc.vector.tensor_add(qt_t[:], qt_t[:], rt[:])
                filler.append((rope_b, True))

            def push_kv(m):
                """K projection + RoPE into kt_s, V projection + transpose into
                vp_s, for chunk m (woven into chunk m's own loop; needed only
                by its diagonal iterations)."""
                st = {}
                proj_items(kvq, m, 1, st, "ps")

                def rope_a():
                    raw = qrp.tile([128, CH], F16, tag="qraw", name=f"qr{m}_k")
                    nc.vector.tensor_copy(raw[:], st["ps"][:])
                    pr = pjp.tile([128, CH], F32, tag="pj", name=f"prm{m}_k")
                    nc.tensor.matmul(pr[:], pm_s[:], raw[:], start=True, stop=True)
                    st["raw"], st["pr"] = raw, pr
                kvq.append((rope_a, True))

                def rope_b():
                    cs_t = xcs[m][1]
                    dst = kt_s[:, CH * m:CH * (m + 1)]
                    nc.vector.tensor_mul(dst, st["raw"][:], cs_t[:, 0:CH])
                    rt = rtp.tile([128, CH], F16, tag="rt")
                    nc.vector.tensor_mul(rt[:], st["pr"][:], cs_t[:, CH:2 * CH])
                    nc.vector.tensor_add(dst, dst, rt[:])
                kvq.append((rope_b, True))
                proj_items(kvq, m, 2, st, "vps")

                def v_evac():
                    vt_t = vtp.tile([128, CH], F16, tag="vt")
                    nc.vector.tensor_copy(vt_t[:], st["vps"][:])
                    st["vt"] = vt_t
                kvq.append((v_evac, True))

                def v_tr(sb_i):
                    def go():
                        tr_ps = pjp.tile([128, 128], F16, tag="pj",
                                         name=f"tr{m}_{sb_i}")
                        nc.tensor.transpose(
                            tr_ps[:], st["vt"][:, 128 * sb_i:128 * (sb_i + 1)],
                            id_s[:])
                        vp = vp_s[(CH // 128) * m + sb_i]
                        nc.vector.tensor_copy(
                            vp[:].rearrange("p (h c) -> p h c", h=2)[:, :, 0:D_K],
                            tr_ps[:].rearrange("p (h c) -> p h c", h=2))
                    return go
                for sb_i in range(CH // 128):
                    kvq.append((v_tr(sb_i), True))

            def pull(n, q=None):
                k = 0
                qq = filler if q is None else q
                safe = True
                while qq and (k < n or not safe):
                    fn, safe = qq.pop(0)
                    fn()
                    k += 1

            def drip(n, q=None):
                qq = woq if q is None else q
                k = 0
                while qq and k < n:
                    qq.pop(0)()
                    k += 1

            def emit_sc(jj, qt_ref, kb):
                rr = kb - 4 * jj
                q0 = 128 * rr if rr > 0 else 0
                sc_t = scp.tile([128, 2 * CH], F32, tag="sc", name=f"sc{jj}_{kb}")
                for h in range(HPC):
                    nc.tensor.matmul(
                        sc_t[:, CH * h + q0:CH * (h + 1)],
                        kt_s[D_K * h:D_K * (h + 1), 128 * kb:128 * (kb + 1)],
                        qt_ref[D_K * h:D_K * (h + 1), q0:CH],
                        start=True, stop=True, tile_position=(D_K * h, 0))
                return sc_t

            # chunk 0 projections run up front
            xcs[0] = (xts0, cs0)
            qt_cur = qtp.tile([EPC, CH], F16, tag="qt", name="qt0")
            push_q0_tile = qt_cur
            st0 = {}
            proj_items(filler, 0, 0, st0, "ps")

            def q0_rope_a():
                raw = qrp.tile([128, CH], F16, tag="qraw", name="qr0_q")
                nc.vector.tensor_copy(raw[:], st0["ps"][:])
                pr = pjp.tile([128, CH], F32, tag="pj", name="prm0_q")
                nc.tensor.matmul(pr[:], pm_s[:], raw[:], start=True, stop=True)
                st0["raw"], st0["pr"] = raw, pr
            filler.append((q0_rope_a, True))

            def q0_rope_b():
                cs_t = xcs[0][1]
                nc.vector.tensor_mul(push_q0_tile[:], st0["raw"][:], cs_t[:, 0:CH])
                rt = rtp.tile([128, CH], F16, tag="rt")
                nc.vector.tensor_mul(rt[:], st0["pr"][:], cs_t[:, CH:2 * CH])
                nc.vector.tensor_add(push_q0_tile[:], push_q0_tile[:], rt[:])
            filler.append((q0_rope_b, True))
            push_kv(0)
            pull(len(filler))
            pull(len(kvq), kvq)

            for j in range(NCH):
                nkb = 4 * (j + 1)
                qt_j = qt_cur
                if j >= 1:
                    push_kv(j)      # this chunk's K/V, due by iteration 4*j
                if j + 1 < NCH:
                    qt_cur = qtp.tile([EPC, CH], F16, tag="qt", name=f"qt{j + 1}")
                    push_q(j + 1, qt_cur)
                total = len(filler)
                done = 0
                kv_total = len(kvq)
                kv_done = 0
                kv_dead = max(1, 4 * j - 2)

                # two psum banks: A = q-subblocks {0,1}, B = {2,3}; each holds
                # four 65-float sub-ranges ordered (q_even h0, q_even h1,
                # q_odd h0, q_odd h1); denominators at col 65k+64
                aoA = aop.tile([128, 512], F32, tag="ao", name=f"aoA{j}")
                aoB = aop.tile([128, 512], F32, tag="ao", name=f"aoB{j}")
                started = [False, False]

                # ---- deferred: normalize, transpose back, W_o, output ----
                ysb = ysp.tile([128, 8 * CH], F16, tag="ysb", name=f"ysb{j}")
                ycnt = [0]

                def norm_pair(ao_ref, a1_tiles, jj, pair):
                    def go():
                        rc_t = rcp.tile([128, 4], F32, tag="rc",
                                        name=f"rc{jj}_{pair}")
                        with nc.allow_low_precision("softmax denom reciprocal"):
                            nc.vector.reciprocal(
                                rc_t[:],
                                ao_ref[:, 0:260].rearrange(
                                    "p (q c) -> p q c", q=4)[:, :, D_K:D_K + 1])
                        for qh in range(4):
                            qsb_l = qh // 2
                            nc.vector.tensor_scalar_mul(
                                a1_tiles[qsb_l][:, D_K * (qh % 2):D_K * (qh % 2 + 1)],
                                ao_ref[:, 65 * qh:65 * qh + D_K],
                                rc_t[:, qh:qh + 1])
                    return go

                def fin_item(qsb, jj, a1_tile, ysb_ref, ycnt_ref):
                    def go():
                        a2_ps = pjp.tile([128, 128], F16, tag="pj",
                                         name=f"a2{jj}_{qsb}")
                        nc.tensor.transpose(a2_ps[:], a1_tile[:], id_s[:])
                        a2_sb = a2p.tile([128, 128], F16, tag="aot2",
                                         name=f"a2s{jj}_{qsb}")
                        nc.vector.tensor_copy(a2_sb[:], a2_ps[:])

                        def wo_half(half):
                            def go2():
                                y_ps = pjp.tile([128, 512], F32, tag="pj",
                                                name=f"y{jj}_{qsb}_{half}")
                                nc.tensor.matmul(
                                    y_ps[:], a2_sb[:],
                                    wot_s[:, 512 * half:512 * (half + 1)],
                                    start=True, stop=True)
                                dst = ysb_ref[:, 1024 * qsb + 512 * half:
                                              1024 * qsb + 512 * (half + 1)]
                                if jj == NCH - 1 and half == 1:
                                    # tail: use the now-idle ACT for half the
                                    # psum evacuations to shorten the ladder
                                    nc.scalar.copy(dst, y_ps[:])
                                else:
                                    nc.vector.tensor_copy(dst, y_ps[:])
                                ycnt_ref[0] += 1
                                if jj == NCH - 1:
                                    if half == 1:
                                        # last chunk: per-qsb DMA fires as soon
                                        # as that 128-row block is complete
                                        nc.sync.dma_start(
                                            y_d[CH * jj + 128 * qsb:
                                                CH * jj + 128 * (qsb + 1),
                                                :].rearrange(
                                                "p (h c) -> p h c", h=2),
                                            ysb_ref[:, 1024 * qsb:
                                                    1024 * (qsb + 1)].rearrange(
                                                "p (h c) -> p h c", h=2))
                                elif ycnt_ref[0] == 8:
                                    nc.sync.dma_start(
                                        y_d[CH * jj:CH * (jj + 1), :].rearrange(
                                            "(q p) (h c) -> p q h c", q=4, h=2),
                                        ysb_ref[:].rearrange(
                                            "p (q h c) -> p q h c", q=4, h=2))
                            return go2
                        woq.append(wo_half(0))
                        woq.append(wo_half(1))
                    return go

                pair_items = []
                for pair, ao_ref in ((0, aoA), (1, aoB)):
                    a1_tiles = [
                        a1p.tile([128, 128], F16, tag="aot1",
                                 name=f"a1{j}_{2 * pair + q}")
                        for q in range(2)]
                    items = [norm_pair(ao_ref, a1_tiles, j, pair)]
                    for qi, qsb in enumerate((2 * pair, 2 * pair + 1)):
                        items.append(fin_item(qsb, j, a1_tiles[qi], ysb, ycnt))
                    pair_items.append(items)

                if j == 0:
                    sc_next = emit_sc(0, qt_j, 0)
                for kb in range(nkb):
                    rr = kb - 4 * j
                    q0 = 128 * rr if rr > 0 else 0
                    sc_t = sc_next
                    et_t = etp.tile([128, 2 * CH], F16, tag="et", name=f"et{j}_{kb}")
                    if rr >= 1:
                        nc.scalar.activation(
                            et_t[:].rearrange("p (h c) -> p h c", h=2)[:, :, q0:CH],
                            sc_t[:].rearrange("p (h c) -> p h c", h=2)[:, :, q0:CH],
                            AF.Exp, scale=0.125)
                    else:
                        nc.scalar.activation(et_t[:], sc_t[:], AF.Exp, scale=0.125)
                    if kb + 1 < nkb:
                        sc_next = emit_sc(j, qt_j, kb + 1)
                    elif j + 1 < NCH:
                        # pre-emit the next chunk's first scores so ACT never
                        # drains across the chunk boundary
                        sc_next = emit_sc(j + 1, qt_cur, 0)
                    # fill PE (and other engines) while ACT runs the exp:
                    # this chunk's K/V first (due before its diagonal), then
                    # next-chunk Q, then deferred Wo work paced to chunk end
                    kv_want = min(kv_total, kv_total * (kb + 1) // kv_dead)
                    pull(kv_want - kv_done, kvq)
                    kv_done = kv_want
                    want = min(total, total * (kb + 1) // max(1, nkb - 2))
                    pull(want - done)
                    done = want
                    drip(len(normq), normq)
                    left = max(1, nkb - kb - 3)
                    drip(max(2, -(-len(woq) // left)) if kb < nkb - 1
                         else len(woq) if j == NCH - 1 else 2)
                    if rr >= 0:
                        # true-diagonal staircase mask on the Pool engine
                        nc.gpsimd.tensor_mul(
                            et_t[:].rearrange("p (h c) -> p h c", h=2)[:, :, q0:q0 + 128],
                            et_t[:].rearrange("p (h c) -> p h c", h=2)[:, :, q0:q0 + 128],
                            msk_s[:].rearrange("p (h c) -> p h c", h=2))
                    # masked (diagonal) q-subblock last: its PV waits on the
                    # Pool mask, so let the other subblocks' PV run first
                    qsbs = [q for q in range(max(0, rr), 4) if q != rr]
                    if rr >= 0:
                        qsbs.append(rr)
                    for qsb in qsbs:
                        ao = aoA if qsb < 2 else aoB
                        bank = 0 if qsb < 2 else 1
                        for h in range(HPC):
                            col0 = 65 * (2 * (qsb % 2) + h)
                            is_first = not started[bank]
                            started[bank] = True
                            is_last = (h == 1) and (qsb == 2 * bank + 1) \
                                and (kb == 4 * j + qsb)
                            nc.tensor.matmul(
                                ao[:, col0:col0 + D_K + 1],
                                et_t[:, CH * h + 128 * qsb:CH * h + 128 * (qsb + 1)],
                                vp_s[kb][:, 65 * h:65 * (h + 1)],
                                start=is_first, stop=is_last, skip_group_check=True)
                    if j == NCH - 1 and kb == 4 * j + 1:
                        for it in pair_items[0]:
                            it()
                        pair_items[0] = []

                # norms must run promptly (they free the psum accumulator
                # banks); the fin/Wo/output work goes to the global backlog
                normq.extend([items[0] for items in pair_items if items])
                for items in pair_items:
                    woq.extend(items[1:])

            pull(len(kvq), kvq)
            pull(len(filler))
            while normq:
                normq.pop(0)()
            while woq:
                woq.pop(0)()
    nc.compile()
    return nc


_PROGRAM = None


def _prep_inputs(x, W_qkv, W_o):
    x2 = np.ascontiguousarray(x.reshape(S, D_MODEL))
    xt = np.ascontiguousarray(x2.T.astype(np.float16))
    cs = _rope_tables()
    pm = _perm_matrix()
    msk2 = _mask2()
    ident = np.eye(128, dtype=np.float16)
    in_maps = []
    for c in range(N_CORES):
        rows = slice(EPC * c, EPC * (c + 1))
        wq = W_qkv[0 * D_MODEL:1 * D_MODEL][rows]
        wk = W_qkv[1 * D_MODEL:2 * D_MODEL][rows]
        wv = W_qkv[2 * D_MODEL:3 * D_MODEL][rows]
        wt = np.concatenate([wq.T, wk.T, wv.T], axis=1)        # [1024, 384]
        wot = W_o[:, rows].T                                   # [128, 1024]
        in_maps.append({
            "xt": xt,
            "wt": np.ascontiguousarray(wt.astype(np.float16)),
            "wot": np.ascontiguousarray(wot.astype(np.float16)),
            "cs": cs,
            "pm": pm,
            "ident": ident,
            "msk2": msk2,
        })
    return in_maps


def kernel(x, W_qkv, W_o):
    global _PROGRAM
    x = np.asarray(x, np.float32)
    W_qkv = np.asarray(W_qkv, np.float32)
    W_o = np.asarray(W_o, np.float32)
    if _PROGRAM is None:
        _PROGRAM = _build_program()
    in_maps = _prep_inputs(x, W_qkv, W_o)
    res = run_bass_kernel_spmd(_PROGRAM, in_maps, core_ids=list(range(N_CORES)))
    acc = np.zeros((S, D_MODEL), np.float32)
    for r in res.results:
        acc += r["y"].astype(np.float32)
    return acc.reshape(1, S, D_MODEL)


# revision 19
# speedup vs baseline: 1.0178x; 1.0178x over previous
"""Trainium2 Bass kernel for causal self-attention with RoPE (fp16 redesign).

Problem: x [1, 4096, 1024], W_qkv [3072, 1024], W_o [1024, 1024], fp32.
  qkv = x @ W_qkv.T; split Q,K,V into 16 heads of d_k=64; RoPE on Q,K;
  causal softmax(Q K^T / 8) @ V; concat heads; @ W_o.T.

Sharding: 2 heads per core across 8 cores (tensor parallel on the head dim).
Each core computes a full [4096, 1024] partial of the output projection in
fp16; host sums the 8 partials.

Layout/engine choices (driven by the TimelineSim cost model, where a matmul
costs out-free-size x cycles_per_row independent of contraction size):
  - everything fp16 (1.0 cycles/row at any width, half the DMA bytes,
    2x DVE modes on all-SBUF 2-byte ops)
  - RoPE rotate-half via one +-1 block-diag permutation matmul per chunk
    (rot(q) = P^T q), killing the two extra "rotated weight" projections
  - attention-out computed TRANSPOSED: aoT[q, d] with lhsT = exp tile,
    rhs = V' -- full 128 output partitions (vs 65 in the [d, q] orientation)
    halves the PV matmul cost, and the softmax denominator (ones column of
    V') lands per-q-PARTITION so normalization is a per-partition scalar mul
  - all four 128-wide q-subblocks x 2 heads of a chunk accumulate into TWO
    psum banks: sub-ranges of a bank share one accumulation group via the
    pending-zero first-touch semantics (start only on the first matmul into
    the bank, stop on the very last)
  - causal masking: only the true diagonal 128x128 staircase blocks get a
    mask multiply, on the otherwise-idle GPSIMD (Pool) engine
  - DMAs merged via multi-dim access patterns (~4 per chunk) because each
    dma_start serializes ~625ns on the single shared HWDGE device
  - projections / W_o / normalization woven into the exp-bound attention
    loop as fine-grained filler items (PE keeps busy while ACT exps)
"""
import numpy as np

import concourse.bacc as bacc
import concourse.mybir as mybir
import concourse.tile as tile
from concourse.bass_utils import run_bass_kernel_spmd

F32 = mybir.dt.float32
F16 = mybir.dt.float16
AF = mybir.ActivationFunctionType

D_MODEL = 1024
N_HEADS = 16
D_K = 64
S = 4096
N_CORES = 8
HPC = N_HEADS // N_CORES      # heads per core = 2
EPC = HPC * D_K               # head dims per core = 128
CH = 512                      # q chunk width
NCH = S // CH                 # 8 chunks
NDT = D_MODEL // 128          # 8 contraction tiles
NKB = S // 128                # 32 key blocks
ROPE_BASE = 10000.0


def _rope_tables():
    inv_freq = (1.0 / (ROPE_BASE ** (np.arange(0, D_K, 2, dtype=np.float64) / D_K)))
    t = np.arange(S, dtype=np.float64)
    freqs = np.outer(t, inv_freq)                              # [S, 32]
    cos = np.concatenate([np.cos(freqs), np.cos(freqs)], 1)    # [S, 64]
    sin = np.concatenate([np.sin(freqs), np.sin(freqs)], 1)
    cosT = np.tile(cos.T, (HPC, 1))                            # [128, S]
    sinT = np.tile(sin.T, (HPC, 1))
    return np.ascontiguousarray(
        np.concatenate([cosT, sinT], axis=1).astype(np.float16))  # [128, 2S]


def _perm_matrix():
    """P with (P^T q)[m] = rot_half(q)[m] per 64-row head block."""
    p = np.zeros((128, 128), dtype=np.float16)
    for h in range(HPC):
        for m in range(D_K):
            if m < 32:
                p[D_K * h + m + 32, D_K * h + m] = -1.0
            else:
                p[D_K * h + m - 32, D_K * h + m] = 1.0
    return p


def _mask2():
    """Staircase valid = (qq >= kk), duplicated for the 2-head strided AP."""
    kk = np.arange(128)[:, None]
    qq = np.arange(128)[None, :]
    m = (qq >= kk).astype(np.float16)
    return np.ascontiguousarray(np.concatenate([m, m], axis=1))  # [128, 256]


def _build_program():
    nc = bacc.Bacc("TRN2", target_bir_lowering=False, debug=False)

    xt_d = nc.dram_tensor("xt", [D_MODEL, S], F16, kind="ExternalInput").ap()
    wt_d = nc.dram_tensor("wt", [D_MODEL, 384], F16, kind="ExternalInput").ap()
    wot_d = nc.dram_tensor("wot", [EPC, D_MODEL], F16, kind="ExternalInput").ap()
    cs_d = nc.dram_tensor("cs", [EPC, 2 * S], F16, kind="ExternalInput").ap()
    pm_d = nc.dram_tensor("pm", [128, 128], F16, kind="ExternalInput").ap()
    id_d = nc.dram_tensor("ident", [128, 128], F16, kind="ExternalInput").ap()
    msk_d = nc.dram_tensor("msk2", [128, 256], F16, kind="ExternalInput").ap()
    y_d = nc.dram_tensor("y", [S, D_MODEL], F16, kind="ExternalOutput").ap()

    with tile.TileContext(nc) as tc:
        with tc.tile_pool(name="const", bufs=1) as cst, \
             tc.tile_pool(name="xts", bufs=2) as xtp, \
             tc.tile_pool(name="cs", bufs=2) as csp, \
             tc.tile_pool(name="qt", bufs=2) as qtp, \
             tc.tile_pool(name="qraw", bufs=2) as qrp, \
             tc.tile_pool(name="rt", bufs=2) as rtp, \
             tc.tile_pool(name="vt", bufs=2) as vtp, \
             tc.tile_pool(name="et", bufs=5) as etp, \
             tc.tile_pool(name="rc", bufs=4) as rcp, \
             tc.tile_pool(name="aot1", bufs=26) as a1p, \
             tc.tile_pool(name="aot2", bufs=8) as a2p, \
             tc.tile_pool(name="ysb", bufs=5) as ysp, \
             tc.tile_pool(name="pj_ps", bufs=2, space="PSUM") as pjp, \
             tc.tile_pool(name="sc_ps", bufs=2, space="PSUM") as scp, \
             tc.tile_pool(name="ao_ps", bufs=2, space="PSUM") as aop:

            # ---- persistent SBUF ----
            wsb = cst.tile([128, NDT * 384], F16, tag="wsb")     # qkv weights
            kt_s = cst.tile([EPC, S], F16, tag="kt")             # rope'd K
            vp_s = [cst.tile([128, 2 * (D_K + 1)], F16, tag=f"vp{i}", name=f"vp{i}")
                    for i in range(NKB)]                          # V' transposed
            pm_s = cst.tile([128, 128], F16, tag="pm")
            id_s = cst.tile([128, 128], F16, tag="id")
            msk_s = cst.tile([128, 256], F16, tag="msk")
            wot_s = cst.tile([EPC, D_MODEL], F16, tag="wot")

            # first DMAs, ordered so the chunk-0 Q projection can start as
            # early as possible: first x/W halves, then the rest
            xts0 = xtp.tile([128, NDT * CH], F16, tag="xts", name="xts0")
            half = NDT // 2
            nc.sync.dma_start(
                xts0[:, 0:half * CH].rearrange("p (dt c) -> p dt c", dt=half),
                xt_d[0:half * 128, 0:CH].rearrange("(dt p) c -> p dt c", dt=half))
            nc.sync.dma_start(
                wsb[:, 0:half * 384].rearrange("p (dt c) -> p dt c", dt=half),
                wt_d[0:half * 128, :].rearrange("(dt p) c -> p dt c", dt=half))
            nc.sync.dma_start(
                xts0[:, half * CH:].rearrange("p (dt c) -> p dt c", dt=half),
                xt_d[half * 128:, 0:CH].rearrange("(dt p) c -> p dt c", dt=half))
            nc.sync.dma_start(
                wsb[:, half * 384:].rearrange("p (dt c) -> p dt c", dt=half),
                wt_d[half * 128:, :].rearrange("(dt p) c -> p dt c", dt=half))
            cs0 = csp.tile([128, 2 * CH], F16, tag="cs", name="cs0")
            nc.sync.dma_start(
                cs0[:].rearrange("p (t c) -> p t c", t=2),
                cs_d[:].rearrange("p (t c) -> p t c", t=2)[:, :, 0:CH])
            nc.sync.dma_start(pm_s[:], pm_d[:])
            nc.sync.dma_start(id_s[:], id_d[:])
            nc.sync.dma_start(msk_s[:], msk_d[:])
            nc.sync.dma_start(wot_s[:], wot_d[:])
            # ones columns of V' (cols 64 and 129), written once
            for i in range(NKB):
                nc.gpsimd.memset(
                    vp_s[i][:].rearrange("p (h c) -> p h c", h=2)[:, :, D_K:D_K + 1],
                    1.0)

            filler = []   # next-chunk Q projection work (due by chunk end)
            kvq = []      # this chunk's K/V work (due before its diagonal)
            normq = []    # psum-bank normalizations (run promptly)
            woq = []      # deferred Wo/output backlog (spent in late chunks)
            xcs = {}      # chunk -> (xts tile, cs tile)

            def push_dma_now(m):
                xts_t = xtp.tile([128, NDT * CH], F16, tag="xts", name=f"xts{m}")
                sl = slice(CH * m, CH * (m + 1))
                nc.sync.dma_start(
                    xts_t[:].rearrange("p (dt c) -> p dt c", dt=NDT),
                    xt_d[:, sl].rearrange("(dt p) c -> p dt c", dt=NDT))
                cs_t = csp.tile([128, 2 * CH], F16, tag="cs", name=f"cs{m}")
                nc.sync.dma_start(
                    cs_t[:].rearrange("p (t c) -> p t c", t=2),
                    cs_d[:].rearrange("p (t c) -> p t c", t=2)[:, :, sl])
                xcs[m] = (xts_t, cs_t)

            def proj_items(q, m, b, st, key):
                """Per-matmul projection items; only the group-closing item is
                a safe stopping point for pull(), so deferred Wo work never
                interleaves into an open psum accumulation group."""
                def mk(dt_i):
                    def go():
                        if dt_i == 0:
                            st[key] = pjp.tile([128, CH], F32, tag="pj",
                                               name=f"pj{m}_{b}")
                        nc.tensor.matmul(
                            st[key][:],
                            wsb[:, 384 * dt_i + 128 * b:384 * dt_i + 128 * (b + 1)],
                            xcs[m][0][:, CH * dt_i:CH * (dt_i + 1)],
                            start=(dt_i == 0), stop=(dt_i == NDT - 1))
                    return go
                for dt_i in range(NDT):
                    q.append((mk(dt_i), dt_i == NDT - 1))

            def push_q(m, qt_t):
                """x/cos-sin DMA + Q projection + RoPE for chunk m."""
                filler.append((lambda: push_dma_now(m), True))
                st = {}
                proj_items(filler, m, 0, st, "ps")

                def rope_a():
                    raw = qrp.tile([128, CH], F16, tag="qraw", name=f"qr{m}_q")
                    nc.vector.tensor_copy(raw[:], st["ps"][:])
                    pr = pjp.tile([128, CH], F32, tag="pj", name=f"prm{m}_q")
                    nc.tensor.matmul(pr[:], pm_s[:], raw[:], start=True, stop=True)
                    st["raw"], st["pr"] = raw, pr
                filler.append((rope_a, True))

                def rope_b():
                    cs_t = xcs[m][1]
                    nc.vector.tensor_mul(qt_t[:], st["raw"][:], cs_t[:, 0:CH])
                    rt = rtp.tile([128, CH], F16, tag="rt")
                    nc.vector.tensor_mul(rt[:], st["pr"][:], cs_t[:, CH:2 * CH])
                    nc.vector.tensor_add(qt_t[:], qt_t[:], rt[:])
                filler.append((rope_b, True))

            def push_kv(m):
                """K projection + RoPE into kt_s, V projection + transpose into
                vp_s, for chunk m (woven into chunk m's own loop; needed only
                by its diagonal iterations)."""
                st = {}
                proj_items(kvq, m, 1, st, "ps")

                def rope_a():
                    raw = qrp.tile([128, CH], F16, tag="qraw", name=f"qr{m}_k")
                    nc.vector.tensor_copy(raw[:], st["ps"][:])
                    pr = pjp.tile([128, CH], F32, tag="pj", name=f"prm{m}_k")
                    nc.tensor.matmul(pr[:], pm_s[:], raw[:], start=True, stop=True)
                    st["raw"], st["pr"] = raw, pr
                kvq.append((rope_a, True))

                def rope_b():
                    cs_t = xcs[m][1]
                    dst = kt_s[:, CH * m:CH * (m + 1)]
                    nc.vector.tensor_mul(dst, st["raw"][:], cs_t[:, 0:CH])
                    rt = rtp.tile([128, CH], F16, tag="rt")
                    nc.vector.tensor_mul(rt[:], st["pr"][:], cs_t[:, CH:2 * CH])
                    nc.vector.tensor_add(dst, dst, rt[:])
                kvq.append((rope_b, True))
                proj_items(kvq, m, 2, st, "vps")

                def v_evac():
                    vt_t = vtp.tile([128, CH], F16, tag="vt")
                    nc.vector.tensor_copy(vt_t[:], st["vps"][:])
                    st["vt"] = vt_t
                kvq.append((v_evac, True))

                def v_tr(sb_i):
                    def go():
                        tr_ps = pjp.tile([128, 128], F16, tag="pj",
                                         name=f"tr{m}_{sb_i}")
                        nc.tensor.transpose(
                            tr_ps[:], st["vt"][:, 128 * sb_i:128 * (sb_i + 1)],
                            id_s[:])
                        vp = vp_s[(CH // 128) * m + sb_i]
                        nc.vector.tensor_copy(
                            vp[:].rearrange("p (h c) -> p h c", h=2)[:, :, 0:D_K],
                            tr_ps[:].rearrange("p (h c) -> p h c", h=2))
                    return go
                for sb_i in range(CH // 128):
                    kvq.append((v_tr(sb_i), True))

            def pull(n, q=None):
                k = 0
                qq = filler if q is None else q
                safe = True
                while qq and (k < n or not safe):
                    fn, safe = qq.pop(0)
                    fn()
                    k += 1

            def drip(n, q=None):
                qq = woq if q is None else q
                k = 0
                while qq and k < n:
                    qq.pop(0)()
                    k += 1

            def emit_sc(jj, qt_ref, kb):
                rr = kb - 4 * jj
                q0 = 128 * rr if rr > 0 else 0
                sc_t = scp.tile([128, 2 * CH], F32, tag="sc", name=f"sc{jj}_{kb}")
                for h in range(HPC):
                    nc.tensor.matmul(
                        sc_t[:, CH * h + q0:CH * (h + 1)],
                        kt_s[D_K * h:D_K * (h + 1), 128 * kb:128 * (kb + 1)],
                        qt_ref[D_K * h:D_K * (h + 1), q0:CH],
                        start=True, stop=True, tile_position=(D_K * h, 0))
                return sc_t

            # chunk 0 projections run up front
            xcs[0] = (xts0, cs0)
            qt_cur = qtp.tile([EPC, CH], F16, tag="qt", name="qt0")
            push_q0_tile = qt_cur
            st0 = {}
            proj_items(filler, 0, 0, st0, "ps")

            def q0_rope_a():
                raw = qrp.tile([128, CH], F16, tag="qraw", name="qr0_q")
                nc.vector.tensor_copy(raw[:], st0["ps"][:])
                pr = pjp.tile([128, CH], F32, tag="pj", name="prm0_q")
                nc.tensor.matmul(pr[:], pm_s[:], raw[:], start=True, stop=True)
                st0["raw"], st0["pr"] = raw, pr
            filler.append((q0_rope_a, True))

            def q0_rope_b():
                cs_t = xcs[0][1]
                nc.vector.tensor_mul(push_q0_tile[:], st0["raw"][:], cs_t[:, 0:CH])
                rt = rtp.tile([128, CH], F16, tag="rt")
                nc.vector.tensor_mul(rt[:], st0["pr"][:], cs_t[:, CH:2 * CH])
                nc.vector.tensor_add(push_q0_tile[:], push_q0_tile[:], rt[:])
            filler.append((q0_rope_b, True))
            push_kv(0)
            pull(len(filler))
            pull(len(kvq), kvq)

            for j in range(NCH):
                nkb = 4 * (j + 1)
                qt_j = qt_cur
                if j >= 1:
                    push_kv(j)      # this chunk's K/V, due by iteration 4*j
                if j + 1 < NCH:
                    qt_cur = qtp.tile([EPC, CH], F16, tag="qt", name=f"qt{j + 1}")
                    push_q(j + 1, qt_cur)
                total = len(filler)
                done = 0
                kv_total = len(kvq)
                kv_done = 0
                kv_dead = max(1, 4 * j - 2)

                # two psum banks: A = q-subblocks {0,1}, B = {2,3}; each holds
                # four 65-float sub-ranges ordered (q_even h0, q_even h1,
                # q_odd h0, q_odd h1); denominators at col 65k+64
                aoA = aop.tile([128, 512], F32, tag="ao", name=f"aoA{j}")
                aoB = aop.tile([128, 512], F32, tag="ao", name=f"aoB{j}")
                started = [False, False]

                # ---- deferred: normalize, transpose back, W_o, output ----
                ysb = ysp.tile([128, 8 * CH], F16, tag="ysb", name=f"ysb{j}")
                ycnt = [0]

                def norm_pair(ao_ref, a1_tiles, jj, pair):
                    def go():
                        rc_t = rcp.tile([128, 4], F32, tag="rc",
                                        name=f"rc{jj}_{pair}")
                        with nc.allow_low_precision("softmax denom reciprocal"):
                            nc.vector.reciprocal(
                                rc_t[:],
                                ao_ref[:, 0:260].rearrange(
                                    "p (q c) -> p q c", q=4)[:, :, D_K:D_K + 1])
                        for qh in range(4):
                            qsb_l = qh // 2
                            nc.vector.tensor_scalar_mul(
                                a1_tiles[qsb_l][:, D_K * (qh % 2):D_K * (qh % 2 + 1)],
                                ao_ref[:, 65 * qh:65 * qh + D_K],
                                rc_t[:, qh:qh + 1])
                    return go

                def fin_item(qsb, jj, a1_tile, ysb_ref, ycnt_ref):
                    def go():
                        a2_ps = pjp.tile([128, 128], F16, tag="pj",
                                         name=f"a2{jj}_{qsb}")
                        nc.tensor.transpose(a2_ps[:], a1_tile[:], id_s[:])
                        a2_sb = a2p.tile([128, 128], F16, tag="aot2",
                                         name=f"a2s{jj}_{qsb}")
                        nc.vector.tensor_copy(a2_sb[:], a2_ps[:])

                        def wo_half(half):
                            def go2():
                                y_ps = pjp.tile([128, 512], F32, tag="pj",
                                                name=f"y{jj}_{qsb}_{half}")
                                nc.tensor.matmul(
                                    y_ps[:], a2_sb[:],
                                    wot_s[:, 512 * half:512 * (half + 1)],
                                    start=True, stop=True)
                                dst = ysb_ref[:, 1024 * qsb + 512 * half:
                                              1024 * qsb + 512 * (half + 1)]
                                if jj == NCH - 1 and half == 1:
                                    # tail: use the now-idle ACT for half the
                                    # psum evacuations to shorten the ladder
                                    nc.scalar.copy(dst, y_ps[:])
                                else:
                                    nc.vector.tensor_copy(dst, y_ps[:])
                                ycnt_ref[0] += 1
                                if jj == NCH - 1:
                                    if half == 1:
                                        # last chunk: per-qsb DMA fires as soon
                                        # as that 128-row block is complete
                                        nc.sync.dma_start(
                                            y_d[CH * jj + 128 * qsb:
                                                CH * jj + 128 * (qsb + 1),
                                                :].rearrange(
                                                "p (h c) -> p h c", h=2),
                                            ysb_ref[:, 1024 * qsb:
                                                    1024 * (qsb + 1)].rearrange(
                                                "p (h c) -> p h c", h=2))
                                elif ycnt_ref[0] == 8:
                                    nc.sync.dma_start(
                                        y_d[CH * jj:CH * (jj + 1), :].rearrange(
                                            "(q p) (h c) -> p q h c", q=4, h=2),
                                        ysb_ref[:].rearrange(
                                            "p (q h c) -> p q h c", q=4, h=2))
                            return go2
                        woq.append(wo_half(0))
                        woq.append(wo_half(1))
                    return go

                pair_items = []
                for pair, ao_ref in ((0, aoA), (1, aoB)):
                    a1_tiles = [
                        a1p.tile([128, 128], F16, tag="aot1",
                                 name=f"a1{j}_{2 * pair + q}")
                        for q in range(2)]
                    items = [norm_pair(ao_ref, a1_tiles, j, pair)]
                    for qi, qsb in enumerate((2 * pair, 2 * pair + 1)):
                        items.append(fin_item(qsb, j, a1_tiles[qi], ysb, ycnt))
                    pair_items.append(items)

                if j == 0:
                    sc_next = emit_sc(0, qt_j, 0)
                for kb in range(nkb):
                    rr = kb - 4 * j
                    q0 = 128 * rr if rr > 0 else 0
                    sc_t = sc_next
                    et_t = etp.tile([128, 2 * CH], F16, tag="et", name=f"et{j}_{kb}")
                    if rr >= 1:
                        nc.scalar.activation(
                            et_t[:].rearrange("p (h c) -> p h c", h=2)[:, :, q0:CH],
                            sc_t[:].rearrange("p (h c) -> p h c", h=2)[:, :, q0:CH],
                            AF.Exp, scale=0.125)
                    else:
                        nc.scalar.activation(et_t[:], sc_t[:], AF.Exp, scale=0.125)
                    if kb + 1 < nkb:
                        sc_next = emit_sc(j, qt_j, kb + 1)
                    elif j + 1 < NCH:
                        # pre-emit the next chunk's first scores so ACT never
                        # drains across the chunk boundary
                        sc_next = emit_sc(j + 1, qt_cur, 0)
                    # fill PE (and other engines) while ACT runs the exp:
                    # this chunk's K/V first (due before its diagonal), then
                    # next-chunk Q, then deferred Wo work paced to chunk end
                    kv_want = min(kv_total, kv_total * (kb + 1) // kv_dead)
                    pull(kv_want - kv_done, kvq)
                    kv_done = kv_want
                    want = min(total, total * (kb + 1) // max(1, nkb - 2))
                    pull(want - done)
                    done = want
                    left = max(1, nkb - kb - 3)
                    drip(max(2, -(-len(woq) // left)) if kb < nkb - 1
                         else len(woq) if j == NCH - 1 else 2)
                    if rr >= 0:
                        # true-diagonal staircase mask on the Pool engine
                        nc.gpsimd.tensor_mul(
                            et_t[:].rearrange("p (h c) -> p h c", h=2)[:, :, q0:q0 + 128],
                            et_t[:].rearrange("p (h c) -> p h c", h=2)[:, :, q0:q0 + 128],
                            msk_s[:].rearrange("p (h c) -> p h c", h=2))
                    # masked (diagonal) q-subblock last: its PV waits on the
                    # Pool mask, so let the other subblocks' PV run first
                    qsbs = [q for q in range(max(0, rr), 4) if q != rr]
                    if rr >= 0:
                        qsbs.append(rr)
                    for qsb in qsbs:
                        ao = aoA if qsb < 2 else aoB
                        bank = 0 if qsb < 2 else 1
                        for h in range(HPC):
                            col0 = 65 * (2 * (qsb % 2) + h)
                            is_first = not started[bank]
                            started[bank] = True
                            is_last = (h == 1) and (qsb == 2 * bank + 1) \
                                and (kb == 4 * j + qsb)
                            nc.tensor.matmul(
                                ao[:, col0:col0 + D_K + 1],
                                et_t[:, CH * h + 128 * qsb:CH * h + 128 * (qsb + 1)],
                                vp_s[kb][:, 65 * h:65 * (h + 1)],
                                start=is_first, stop=is_last, skip_group_check=True)
                    if j == NCH - 1 and kb == 4 * j + 1:
                        for it in pair_items[0]:
                            it()
                        pair_items[0] = []

                # norms first so the next chunk's PV matmuls get their psum
                # accumulator banks back quickly
                woq.extend([items[0] for items in pair_items if items])
                for items in pair_items:
                    woq.extend(items[1:])

            pull(len(kvq), kvq)
            pull(len(filler))
            while woq:
                woq.pop(0)()
    nc.compile()
    return nc


_PROGRAM = None


def _prep_inputs(x, W_qkv, W_o):
    x2 = np.ascontiguousarray(x.reshape(S, D_MODEL))
    xt = np.ascontiguousarray(x2.T.astype(np.float16))
    cs = _rope_tables()
    pm = _perm_matrix()
    msk2 = _mask2()
    ident = np.eye(128, dtype=np.float16)
    in_maps = []
    for c in range(N_CORES):
        rows = slice(EPC * c, EPC * (c + 1))
        wq = W_qkv[0 * D_MODEL:1 * D_MODEL][rows]
        wk = W_qkv[1 * D_MODEL:2 * D_MODEL][rows]
        wv = W_qkv[2 * D_MODEL:3 * D_MODEL][rows]
        wt = np.concatenate([wq.T, wk.T, wv.T], axis=1)        # [1024, 384]
        wot = W_o[:, rows].T                                   # [128, 1024]
        in_maps.append({
            "xt": xt,
            "wt": np.ascontiguousarray(wt.astype(np.float16)),
            "wot": np.ascontiguousarray(wot.astype(np.float16)),
            "cs": cs,
            "pm": pm,
            "ident": ident,
            "msk2": msk2,
        })
    return in_maps


def kernel(x, W_qkv, W_o):
    global _PROGRAM
    x = np.asarray(x, np.float32)
    W_qkv = np.asarray(W_qkv, np.float32)
    W_o = np.asarray(W_o, np.float32)
    if _PROGRAM is None:
        _PROGRAM = _build_program()
    in_maps = _prep_inputs(x, W_qkv, W_o)
    res = run_bass_kernel_spmd(_PROGRAM, in_maps, core_ids=list(range(N_CORES)))
    acc = np.zeros((S, D_MODEL), np.float32)
    for r in res.results:
        acc += r["y"].astype(np.float32)
    return acc.reshape(1, S, D_MODEL)


# revision 26
# speedup vs baseline: 1.0356x; 1.0175x over previous
"""Trainium2 Bass kernel for causal self-attention with RoPE (fp16 redesign).

Problem: x [1, 4096, 1024], W_qkv [3072, 1024], W_o [1024, 1024], fp32.
  qkv = x @ W_qkv.T; split Q,K,V into 16 heads of d_k=64; RoPE on Q,K;
  causal softmax(Q K^T / 8) @ V; concat heads; @ W_o.T.

Sharding: 2 heads per core across 8 cores (tensor parallel on the head dim).
Each core computes a full [4096, 1024] partial of the output projection in
fp16; host sums the 8 partials.

Layout/engine choices (driven by the TimelineSim cost model, where a matmul
costs out-free-size x cycles_per_row independent of contraction size):
  - everything fp16 (1.0 cycles/row at any width, half the DMA bytes,
    2x DVE modes on all-SBUF 2-byte ops)
  - RoPE rotate-half via one +-1 block-diag permutation matmul per chunk
    (rot(q) = P^T q), killing the two extra "rotated weight" projections
  - attention-out computed TRANSPOSED: aoT[q, d] with lhsT = exp tile,
    rhs = V' -- full 128 output partitions (vs 65 in the [d, q] orientation)
    halves the PV matmul cost, and the softmax denominator (ones column of
    V') lands per-q-PARTITION so normalization is a per-partition scalar mul
  - all four 128-wide q-subblocks x 2 heads of a chunk accumulate into TWO
    psum banks: sub-ranges of a bank share one accumulation group via the
    pending-zero first-touch semantics (start only on the first matmul into
    the bank, stop on the very last)
  - causal masking: only the true diagonal 128x128 staircase blocks get a
    mask multiply, on the otherwise-idle GPSIMD (Pool) engine
  - DMAs merged via multi-dim access patterns (~4 per chunk) because each
    dma_start serializes ~625ns on the single shared HWDGE device
  - projections / W_o / normalization woven into the exp-bound attention
    loop as fine-grained filler items (PE keeps busy while ACT exps)
"""
import numpy as np

import concourse.bacc as bacc
import concourse.mybir as mybir
import concourse.tile as tile
from concourse.bass_utils import run_bass_kernel_spmd

F32 = mybir.dt.float32
F16 = mybir.dt.float16
AF = mybir.ActivationFunctionType

D_MODEL = 1024
N_HEADS = 16
D_K = 64
S = 4096
N_CORES = 8
HPC = N_HEADS // N_CORES      # heads per core = 2
EPC = HPC * D_K               # head dims per core = 128
CH = 512                      # q chunk width
NCH = S // CH                 # 8 chunks
NDT = D_MODEL // 128          # 8 contraction tiles
NKB = S // 128                # 32 key blocks
ROPE_BASE = 10000.0


def _rope_tables():
    inv_freq = (1.0 / (ROPE_BASE ** (np.arange(0, D_K, 2, dtype=np.float64) / D_K)))
    t = np.arange(S, dtype=np.float64)
    freqs = np.outer(t, inv_freq)                              # [S, 32]
    cos = np.concatenate([np.cos(freqs), np.cos(freqs)], 1)    # [S, 64]
    sin = np.concatenate([np.sin(freqs), np.sin(freqs)], 1)
    cosT = np.tile(cos.T, (HPC, 1))                            # [128, S]
    sinT = np.tile(sin.T, (HPC, 1))
    return np.ascontiguousarray(
        np.concatenate([cosT, sinT], axis=1).astype(np.float16))  # [128, 2S]


def _perm_matrix():
    """P with (P^T q)[m] = rot_half(q)[m] per 64-row head block."""
    p = np.zeros((128, 128), dtype=np.float16)
    for h in range(HPC):
        for m in range(D_K):
            if m < 32:
                p[D_K * h + m + 32, D_K * h + m] = -1.0
            else:
                p[D_K * h + m - 32, D_K * h + m] = 1.0
    return p


def _mask2():
    """Staircase valid = (qq >= kk), duplicated for the 2-head strided AP."""
    kk = np.arange(128)[:, None]
    qq = np.arange(128)[None, :]
    m = (qq >= kk).astype(np.float16)
    return np.ascontiguousarray(np.concatenate([m, m], axis=1))  # [128, 256]


def _build_program():
    nc = bacc.Bacc("TRN2", target_bir_lowering=False, debug=False)

    xt_d = nc.dram_tensor("xt", [D_MODEL, S], F16, kind="ExternalInput").ap()
    wt_d = nc.dram_tensor("wt", [D_MODEL, 384], F16, kind="ExternalInput").ap()
    wot_d = nc.dram_tensor("wot", [EPC, D_MODEL], F16, kind="ExternalInput").ap()
    cs_d = nc.dram_tensor("cs", [EPC, 2 * S], F16, kind="ExternalInput").ap()
    pm_d = nc.dram_tensor("pm", [128, 128], F16, kind="ExternalInput").ap()
    id_d = nc.dram_tensor("ident", [128, 128], F16, kind="ExternalInput").ap()
    msk_d = nc.dram_tensor("msk2", [128, 256], F16, kind="ExternalInput").ap()
    y_d = nc.dram_tensor("y", [S, D_MODEL], F16, kind="ExternalOutput").ap()

    with tile.TileContext(nc) as tc:
        with tc.tile_pool(name="const", bufs=1) as cst, \
             tc.tile_pool(name="xts", bufs=2) as xtp, \
             tc.tile_pool(name="cs", bufs=2) as csp, \
             tc.tile_pool(name="qt", bufs=2) as qtp, \
             tc.tile_pool(name="qraw", bufs=3) as qrp, \
             tc.tile_pool(name="rt", bufs=3) as rtp, \
             tc.tile_pool(name="vt", bufs=3) as vtp, \
             tc.tile_pool(name="et", bufs=8) as etp, \
             tc.tile_pool(name="rc", bufs=4) as rcp, \
             tc.tile_pool(name="aot1", bufs=26) as a1p, \
             tc.tile_pool(name="aot2", bufs=8) as a2p, \
             tc.tile_pool(name="ysb", bufs=5) as ysp, \
             tc.tile_pool(name="pj_ps", bufs=2, space="PSUM") as pjp, \
             tc.tile_pool(name="sc_ps", bufs=2, space="PSUM") as scp, \
             tc.tile_pool(name="ao_ps", bufs=2, space="PSUM") as aop:

            # ---- persistent SBUF ----
            wsb = cst.tile([128, NDT * 384], F16, tag="wsb")     # qkv weights
            kt_s = cst.tile([EPC, S], F16, tag="kt")             # rope'd K
            vp_s = [cst.tile([128, 2 * (D_K + 1)], F16, tag=f"vp{i}", name=f"vp{i}")
                    for i in range(NKB)]                          # V' transposed
            pm_s = cst.tile([128, 128], F16, tag="pm")
            id_s = cst.tile([128, 128], F16, tag="id")
            msk_s = cst.tile([128, 256], F16, tag="msk")
            wot_s = cst.tile([EPC, D_MODEL], F16, tag="wot")

            # first DMAs, ordered for the chunk-0 critical chain: a tiny
            # dt0 x/W slice first (the first projection matmul's inputs),
            # then rope tables (needed ~3us in), then the bulk
            xts0 = xtp.tile([128, NDT * CH], F16, tag="xts", name="xts0")
            half = NDT // 2
            nc.sync.dma_start(xts0[:, 0:CH], xt_d[0:128, 0:CH])
            nc.sync.dma_start(wsb[:, 0:384], wt_d[0:128, :])
            cs0 = csp.tile([128, 2 * CH], F16, tag="cs", name="cs0")
            nc.sync.dma_start(
                cs0[:].rearrange("p (t c) -> p t c", t=2),
                cs_d[:].rearrange("p (t c) -> p t c", t=2)[:, :, 0:CH])
            nc.sync.dma_start(pm_s[:], pm_d[:])
            nc.sync.dma_start(
                xts0[:, CH:half * CH].rearrange("p (dt c) -> p dt c", dt=half - 1),
                xt_d[128:half * 128, 0:CH].rearrange(
                    "(dt p) c -> p dt c", dt=half - 1))
            nc.sync.dma_start(
                wsb[:, 384:half * 384].rearrange("p (dt c) -> p dt c", dt=half - 1),
                wt_d[128:half * 128, :].rearrange("(dt p) c -> p dt c", dt=half - 1))
            nc.sync.dma_start(
                xts0[:, half * CH:].rearrange("p (dt c) -> p dt c", dt=half),
                xt_d[half * 128:, 0:CH].rearrange("(dt p) c -> p dt c", dt=half))
            nc.sync.dma_start(
                wsb[:, half * 384:].rearrange("p (dt c) -> p dt c", dt=half),
                wt_d[half * 128:, :].rearrange("(dt p) c -> p dt c", dt=half))
            nc.sync.dma_start(id_s[:], id_d[:])
            nc.sync.dma_start(msk_s[:], msk_d[:])
            nc.sync.dma_start(wot_s[:], wot_d[:])
            # ones columns of V' (cols 64 and 129), written once
            for i in range(NKB):
                nc.gpsimd.memset(
                    vp_s[i][:].rearrange("p (h c) -> p h c", h=2)[:, :, D_K:D_K + 1],
                    1.0)

            filler = []   # next-chunk Q projection work (due by chunk end)
            kvq = []      # this chunk's K/V work (due before its diagonal)
            normq = []    # psum-bank normalizations (run promptly)
            woq = []      # deferred Wo/output backlog (spent in late chunks)
            xcs = {}      # chunk -> (xts tile, cs tile)

            def push_dma_now(m):
                xts_t = xtp.tile([128, NDT * CH], F16, tag="xts", name=f"xts{m}")
                sl = slice(CH * m, CH * (m + 1))
                nc.sync.dma_start(
                    xts_t[:].rearrange("p (dt c) -> p dt c", dt=NDT),
                    xt_d[:, sl].rearrange("(dt p) c -> p dt c", dt=NDT))
                cs_t = csp.tile([128, 2 * CH], F16, tag="cs", name=f"cs{m}")
                nc.sync.dma_start(
                    cs_t[:].rearrange("p (t c) -> p t c", t=2),
                    cs_d[:].rearrange("p (t c) -> p t c", t=2)[:, :, sl])
                xcs[m] = (xts_t, cs_t)

            def proj_items(q, m, b, st, key):
                """Per-matmul projection items; only the group-closing item is
                a safe stopping point for pull(), so deferred Wo work never
                interleaves into an open psum accumulation group."""
                def mk(dt_i):
                    def go():
                        if dt_i == 0:
                            st[key] = pjp.tile([128, CH], F32, tag="pj",
                                               name=f"pj{m}_{b}")
                        nc.tensor.matmul(
                            st[key][:],
                            wsb[:, 384 * dt_i + 128 * b:384 * dt_i + 128 * (b + 1)],
                            xcs[m][0][:, CH * dt_i:CH * (dt_i + 1)],
                            start=(dt_i == 0), stop=(dt_i == NDT - 1))
                    return go
                for dt_i in range(NDT):
                    q.append((mk(dt_i), dt_i == NDT - 1))

            def push_q(m, qt_t):
                """x/cos-sin DMA + Q projection + RoPE for chunk m."""
                filler.append((lambda: push_dma_now(m), True))
                st = {}
                proj_items(filler, m, 0, st, "ps")

                def rope_a():
                    raw = qrp.tile([128, CH], F16, tag="qraw", name=f"qr{m}_q")
                    nc.vector.tensor_copy(raw[:], st["ps"][:])
                    pr = pjp.tile([128, CH], F32, tag="pj", name=f"prm{m}_q")
                    nc.tensor.matmul(pr[:], pm_s[:], raw[:], start=True, stop=True)
                    st["raw"], st["pr"] = raw, pr
                filler.append((rope_a, True))

                def rope_b():
                    cs_t = xcs[m][1]
                    nc.vector.tensor_mul(qt_t[:], st["raw"][:], cs_t[:, 0:CH])
                    rt = rtp.tile([128, CH], F16, tag="rt")
                    nc.vector.tensor_mul(rt[:], st["pr"][:], cs_t[:, CH:2 * CH])
                    nc.vector.tensor_add(qt_t[:], qt_t[:], rt[:])
                filler.append((rope_b, True))

            def push_kv(m):
                """K projection + RoPE into kt_s, V projection + transpose into
                vp_s, for chunk m (woven into chunk m's own loop; needed only
                by its diagonal iterations)."""
                st = {}
                proj_items(kvq, m, 1, st, "ps")

                def rope_a():
                    raw = qrp.tile([128, CH], F16, tag="qraw", name=f"qr{m}_k")
                    nc.vector.tensor_copy(raw[:], st["ps"][:])
                    pr = pjp.tile([128, CH], F32, tag="pj", name=f"prm{m}_k")
                    nc.tensor.matmul(pr[:], pm_s[:], raw[:], start=True, stop=True)
                    st["raw"], st["pr"] = raw, pr
                kvq.append((rope_a, True))

                def rope_b():
                    cs_t = xcs[m][1]
                    dst = kt_s[:, CH * m:CH * (m + 1)]
                    nc.vector.tensor_mul(dst, st["raw"][:], cs_t[:, 0:CH])
                    rt = rtp.tile([128, CH], F16, tag="rt")
                    nc.vector.tensor_mul(rt[:], st["pr"][:], cs_t[:, CH:2 * CH])
                    nc.vector.tensor_add(dst, dst, rt[:])
                kvq.append((rope_b, True))
                proj_items(kvq, m, 2, st, "vps")

                def v_evac():
                    vt_t = vtp.tile([128, CH], F16, tag="vt")
                    nc.vector.tensor_copy(vt_t[:], st["vps"][:])
                    st["vt"] = vt_t
                kvq.append((v_evac, True))

                def v_tr(sb_i):
                    def go():
                        tr_ps = pjp.tile([128, 128], F16, tag="pj",
                                         name=f"tr{m}_{sb_i}")
                        nc.tensor.transpose(
                            tr_ps[:], st["vt"][:, 128 * sb_i:128 * (sb_i + 1)],
                            id_s[:])
                        vp = vp_s[(CH // 128) * m + sb_i]
                        nc.vector.tensor_copy(
                            vp[:].rearrange("p (h c) -> p h c", h=2)[:, :, 0:D_K],
                            tr_ps[:].rearrange("p (h c) -> p h c", h=2))
                    return go
                for sb_i in range(CH // 128):
                    kvq.append((v_tr(sb_i), True))

            def pull(n, q=None):
                k = 0
                qq = filler if q is None else q
                safe = True
                while qq and (k < n or not safe):
                    fn, safe = qq.pop(0)
                    fn()
                    k += 1

            def drip(n, q=None):
                qq = woq if q is None else q
                k = 0
                while qq and k < n:
                    qq.pop(0)()
                    k += 1

            def emit_sc(jj, qt_ref, kb):
                rr = kb - 4 * jj
                q0 = 128 * rr if rr > 0 else 0
                sc_t = scp.tile([128, 2 * CH], F32, tag="sc", name=f"sc{jj}_{kb}")
                for h in range(HPC):
                    nc.tensor.matmul(
                        sc_t[:, CH * h + q0:CH * (h + 1)],
                        kt_s[D_K * h:D_K * (h + 1), 128 * kb:128 * (kb + 1)],
                        qt_ref[D_K * h:D_K * (h + 1), q0:CH],
                        start=True, stop=True, tile_position=(D_K * h, 0))
                return sc_t

            # chunk 0 projections run up front
            xcs[0] = (xts0, cs0)
            qt_cur = qtp.tile([EPC, CH], F16, tag="qt", name="qt0")
            push_q0_tile = qt_cur
            st0 = {}
            proj_items(filler, 0, 0, st0, "ps")

            def q0_rope_a():
                raw = qrp.tile([128, CH], F16, tag="qraw", name="qr0_q")
                nc.vector.tensor_copy(raw[:], st0["ps"][:])
                pr = pjp.tile([128, CH], F32, tag="pj", name="prm0_q")
                nc.tensor.matmul(pr[:], pm_s[:], raw[:], start=True, stop=True)
                st0["raw"], st0["pr"] = raw, pr
            filler.append((q0_rope_a, True))

            def q0_rope_b():
                cs_t = xcs[0][1]
                nc.vector.tensor_mul(push_q0_tile[:], st0["raw"][:], cs_t[:, 0:CH])
                rt = rtp.tile([128, CH], F16, tag="rt")
                nc.vector.tensor_mul(rt[:], st0["pr"][:], cs_t[:, CH:2 * CH])
                nc.vector.tensor_add(push_q0_tile[:], push_q0_tile[:], rt[:])
            filler.append((q0_rope_b, True))
            push_kv(0)
            pull(len(filler))
            pull(len(kvq), kvq)

            for j in range(NCH):
                nkb = 4 * (j + 1)
                qt_j = qt_cur
                if j >= 1:
                    push_kv(j)      # this chunk's K/V, due by iteration 4*j
                if j + 1 < NCH:
                    qt_cur = qtp.tile([EPC, CH], F16, tag="qt", name=f"qt{j + 1}")
                    push_q(j + 1, qt_cur)
                total = len(filler)
                done = 0
                kv_total = len(kvq)
                kv_done = 0
                kv_dead = max(1, 4 * j - 1)

                # two psum banks: A = q-subblocks {0,1}, B = {2,3}; each holds
                # four 65-float sub-ranges ordered (q_even h0, q_even h1,
                # q_odd h0, q_odd h1); denominators at col 65k+64
                aoA = aop.tile([128, 512], F32, tag="ao", name=f"aoA{j}")
                aoB = aop.tile([128, 512], F32, tag="ao", name=f"aoB{j}")
                started = [False, False]

                # ---- deferred: normalize, transpose back, W_o, output ----
                ysb = ysp.tile([128, 8 * CH], F16, tag="ysb", name=f"ysb{j}")
                ycnt = [0]

                def norm_pair(ao_ref, a1_tiles, jj, pair):
                    def go():
                        rc_t = rcp.tile([128, 4], F32, tag="rc",
                                        name=f"rc{jj}_{pair}")
                        with nc.allow_low_precision("softmax denom reciprocal"):
                            nc.vector.reciprocal(
                                rc_t[:],
                                ao_ref[:, 0:260].rearrange(
                                    "p (q c) -> p q c", q=4)[:, :, D_K:D_K + 1])
                        for qh in range(4):
                            qsb_l = qh // 2
                            nc.vector.tensor_scalar_mul(
                                a1_tiles[qsb_l][:, D_K * (qh % 2):D_K * (qh % 2 + 1)],
                                ao_ref[:, 65 * qh:65 * qh + D_K],
                                rc_t[:, qh:qh + 1])
                    return go

                def fin_item(qsb, jj, a1_tile, ysb_ref, ycnt_ref):
                    def go():
                        a2_ps = pjp.tile([128, 128], F16, tag="pj",
                                         name=f"a2{jj}_{qsb}")
                        nc.tensor.transpose(a2_ps[:], a1_tile[:], id_s[:])
                        a2_sb = a2p.tile([128, 128], F16, tag="aot2",
                                         name=f"a2s{jj}_{qsb}")
                        nc.vector.tensor_copy(a2_sb[:], a2_ps[:])

                        def wo_half(half):
                            def go2():
                                y_ps = pjp.tile([128, 512], F32, tag="pj",
                                                name=f"y{jj}_{qsb}_{half}")
                                nc.tensor.matmul(
                                    y_ps[:], a2_sb[:],
                                    wot_s[:, 512 * half:512 * (half + 1)],
                                    start=True, stop=True)
                                dst = ysb_ref[:, 1024 * qsb + 512 * half:
                                              1024 * qsb + 512 * (half + 1)]
                                if jj == NCH - 1 and half == 1:
                                    # tail: use the now-idle ACT for half the
                                    # psum evacuations to shorten the ladder
                                    nc.scalar.copy(dst, y_ps[:])
                                else:
                                    nc.vector.tensor_copy(dst, y_ps[:])
                                ycnt_ref[0] += 1
                                if jj == NCH - 1:
                                    if half == 1:
                                        # last chunk: per-qsb DMA fires as soon
                                        # as that 128-row block is complete
                                        nc.sync.dma_start(
                                            y_d[CH * jj + 128 * qsb:
                                                CH * jj + 128 * (qsb + 1),
                                                :].rearrange(
                                                "p (h c) -> p h c", h=2),
                                            ysb_ref[:, 1024 * qsb:
                                                    1024 * (qsb + 1)].rearrange(
                                                "p (h c) -> p h c", h=2))
                                elif ycnt_ref[0] == 8:
                                    nc.sync.dma_start(
                                        y_d[CH * jj:CH * (jj + 1), :].rearrange(
                                            "(q p) (h c) -> p q h c", q=4, h=2),
                                        ysb_ref[:].rearrange(
                                            "p (q h c) -> p q h c", q=4, h=2))
                            return go2
                        woq.append(wo_half(0))
                        woq.append(wo_half(1))
                    return go

                pair_items = []
                for pair, ao_ref in ((0, aoA), (1, aoB)):
                    a1_tiles = [
                        a1p.tile([128, 128], F16, tag="aot1",
                                 name=f"a1{j}_{2 * pair + q}")
                        for q in range(2)]
                    items = [norm_pair(ao_ref, a1_tiles, j, pair)]
                    for qi, qsb in enumerate((2 * pair, 2 * pair + 1)):
                        items.append(fin_item(qsb, j, a1_tiles[qi], ysb, ycnt))
                    pair_items.append(items)

                if j == 0:
                    sc_next = emit_sc(0, qt_j, 0)
                for kb in range(nkb):
                    rr = kb - 4 * j
                    q0 = 128 * rr if rr > 0 else 0
                    sc_t = sc_next
                    et_t = etp.tile([128, 2 * CH], F16, tag="et", name=f"et{j}_{kb}")
                    if rr >= 1:
                        nc.scalar.activation(
                            et_t[:].rearrange("p (h c) -> p h c", h=2)[:, :, q0:CH],
                            sc_t[:].rearrange("p (h c) -> p h c", h=2)[:, :, q0:CH],
                            AF.Exp, scale=0.125)
                    else:
                        nc.scalar.activation(et_t[:], sc_t[:], AF.Exp, scale=0.125)
                    if kb + 1 < nkb:
                        sc_next = emit_sc(j, qt_j, kb + 1)
                    elif j + 1 < NCH:
                        # pre-emit the next chunk's first scores so ACT never
                        # drains across the chunk boundary
                        sc_next = emit_sc(j + 1, qt_cur, 0)
                    # fill PE (and other engines) while ACT runs the exp:
                    # this chunk's K/V first (due before its diagonal), then
                    # next-chunk Q, then deferred Wo work paced to chunk end
                    kv_want = min(kv_total, kv_total * (kb + 1) // kv_dead)
                    pull(kv_want - kv_done, kvq)
                    kv_done = kv_want
                    want = min(total, total * (kb + 1) // max(1, nkb - 1))
                    pull(want - done)
                    done = want
                    # spread deferred Wo over ~2 chunks of iterations so the
                    # ACT-bound late chunks keep PE fed; norms (queue front)
                    # still drain in the first iterations
                    spill = 4 * (2 * j + 5) if j < NCH - 2 else                         (4 * (j + 2) if j < NCH - 1 else 0)
                    left = max(1, nkb - kb - 3 + spill)
                    drip(2 if kb < 2
                         else max(1, -(-len(woq) // left)) if kb < nkb - 1
                         else len(woq) if j == NCH - 1 else 1)
                    if rr >= 0:
                        # true-diagonal staircase mask on the Pool engine
                        nc.gpsimd.tensor_mul(
                            et_t[:].rearrange("p (h c) -> p h c", h=2)[:, :, q0:q0 + 128],
                            et_t[:].rearrange("p (h c) -> p h c", h=2)[:, :, q0:q0 + 128],
                            msk_s[:].rearrange("p (h c) -> p h c", h=2))
                    # masked (diagonal) q-subblock last: its PV waits on the
                    # Pool mask, so let the other subblocks' PV run first
                    qsbs = [q for q in range(max(0, rr), 4) if q != rr]
                    if rr >= 0:
                        qsbs.append(rr)
                    for qsb in qsbs:
                        ao = aoA if qsb < 2 else aoB
                        bank = 0 if qsb < 2 else 1
                        for h in range(HPC):
                            col0 = 65 * (2 * (qsb % 2) + h)
                            is_first = not started[bank]
                            started[bank] = True
                            is_last = (h == 1) and (qsb == 2 * bank + 1) \
                                and (kb == 4 * j + qsb)
                            nc.tensor.matmul(
                                ao[:, col0:col0 + D_K + 1],
                                et_t[:, CH * h + 128 * qsb:CH * h + 128 * (qsb + 1)],
                                vp_s[kb][:, 65 * h:65 * (h + 1)],
                                start=is_first, stop=is_last, skip_group_check=True)
                    if j == NCH - 1 and kb == 4 * j + 1:
                        for it in pair_items[0]:
                            it()
                        pair_items[0] = []

                # norms first so the next chunk's PV matmuls get their psum
                # accumulator banks back quickly
                woq.extend([items[0] for items in pair_items if items])
                for items in pair_items:
                    woq.extend(items[1:])

            pull(len(kvq), kvq)
            pull(len(filler))
            while woq:
                woq.pop(0)()
    nc.compile()
    return nc


_PROGRAM = None


def _prep_inputs(x, W_qkv, W_o):
    x2 = np.ascontiguousarray(x.reshape(S, D_MODEL))
    xt = np.ascontiguousarray(x2.T.astype(np.float16))
    cs = _rope_tables()
    pm = _perm_matrix()
    msk2 = _mask2()
    ident = np.eye(128, dtype=np.float16)
    in_maps = []
    for c in range(N_CORES):
        rows = slice(EPC * c, EPC * (c + 1))
        wq = W_qkv[0 * D_MODEL:1 * D_MODEL][rows]
        wk = W_qkv[1 * D_MODEL:2 * D_MODEL][rows]
        wv = W_qkv[2 * D_MODEL:3 * D_MODEL][rows]
        wt = np.concatenate([wq.T, wk.T, wv.T], axis=1)        # [1024, 384]
        wot = W_o[:, rows].T                                   # [128, 1024]
        in_maps.append({
            "xt": xt,
            "wt": np.ascontiguousarray(wt.astype(np.float16)),
            "wot": np.ascontiguousarray(wot.astype(np.float16)),
            "cs": cs,
            "pm": pm,
            "ident": ident,
            "msk2": msk2,
        })
    return in_maps


def kernel(x, W_qkv, W_o):
    global _PROGRAM
    x = np.asarray(x, np.float32)
    W_qkv = np.asarray(W_qkv, np.float32)
    W_o = np.asarray(W_o, np.float32)
    if _PROGRAM is None:
        _PROGRAM = _build_program()
    in_maps = _prep_inputs(x, W_qkv, W_o)
    res = run_bass_kernel_spmd(_PROGRAM, in_maps, core_ids=list(range(N_CORES)))
    acc = np.zeros((S, D_MODEL), np.float32)
    for r in res.results:
        acc += r["y"].astype(np.float32)
    return acc.reshape(1, S, D_MODEL)
